# revision 34
# baseline (speedup 1.0000x reference)
"""Trainium2 Bass kernel for BinaryMaskEdgeSmoothing.

The reference per-pixel computation (two 3x3 convs + sigmoid blend +
threshold) collapses, for binary inputs, to a single linear threshold
function of the 3x3 neighborhood (verified against the f32 reference
over all 512 neighborhoods):

    out = [ conv3x3(x, K) > 22.5 ],  K = [[3,5,3],[5,12,5],[3,5,3]]

Device decomposition (8 NeuronCores, data-parallel over B*C=64 images,
8 images per core, row-tiles of 128 rows with stride 126):

  * Two adjacent pixels are packed into one fp16 element on the host,
    plus a pre-summed odd-pixel fp8 stream (1.5 B/px total input):
      Q[r, c]  = 64*x[r, 2c] + x[r, 2c+1]       (values {0,1,64,65})
      P2[r, c] = x[r, 2c-1] + x[r, 2c+1]        (values {0,1,2}, fp8)
    Three banded-matrix matmuls (TensorEngine; vertical taps in the
    128x128 band, vertical profiles a=[3,5,3], c=[5,12,5]) compute BOTH
    pixels' conv values in one PSUM element:
      psi = band(c + a/64)@Q + band(64a)@P2 + band(a/64)@Q[:, 1:]
          = 64*A + B + eps   (A = conv at even px, B = odd px)
    where eps in [0, 0.35) is a deliberate fractional residue (the a/64
    rider of each Q pass lands on the neighbouring odd pixel) plus
    <~0.1 of fp22 product rounding; both vanish in the uint16 round.
  * psi is converted to uint16 (DVE/ACT alternating) and DMA'd out; the
    host applies both thresholds:
      even bit = (psi >> 6) > 22,  odd bit = (psi & 63) > 22
    (A, B in [0,44], so the bases never interact; arithmetic exact.)
  * Rows 1009..1023 of all 8 images are batched into one final tile
    with block-diagonal band matrices.
"""

import numpy as np

import concourse.bass as bass
import concourse.bacc as bacc
import concourse.mybir as mybir
import concourse.tile as tile
from concourse.bass_utils import run_bass_kernel_spmd

Op = mybir.AluOpType
F32 = mybir.dt.float32
FP16 = mybir.dt.float16
FP8 = mybir.dt.float8e4
U16 = mybir.dt.uint16

N_CORES = 8
B_PER_CORE = 8
H = 1024
W = 1024
NP = W // 2          # pair columns
N_MAIN = 8           # main row-units per image (stride 126, 128 rows)
TAIL_S = 16          # input rows per image in the tail unit
N_UNITS = B_PER_CORE * N_MAIN + 1

BASE = 64.0

# Truth table of the reference computation over neighborhood classes
# (center c, adjacent count a, diagonal count d), derived by running the
# f32 jax reference over all 512 binary 3x3 neighborhoods with the
# canonical lap/gauss kernels. Index = c*25 + a*5 + d.
_TRUTH = np.array([int(ch) for ch in (
    "00000" "00000" "00000" "00011" "01111"   # c=0, a=0..4, d=0..4
    "00001" "00111" "01111" "11111" "11111"   # c=1
)], dtype=np.int64)
_LAP = np.array([[-1., -1., -1.], [-1., 8., -1.], [-1., -1., -1.]],
                np.float32)
_GAU = (np.array([[1., 2., 1.], [2., 4., 2.], [1., 2., 1.]],
                 np.float32) / 16.0)


def _check_kernels(lap, gauss):
    """The LTF below encodes the reference's decisions for the canonical
    lap/gauss kernels (verified per-class through the f32 jax reference,
    including the two classes whose exact value is within 1e-8 of 0.5,
    where f32 rounding decides). Different conv kernels would need a
    re-derived table, so refuse them loudly."""
    lap = np.asarray(lap, np.float32).reshape(3, 3)
    gauss = np.asarray(gauss, np.float32).reshape(3, 3)
    if not (np.array_equal(lap, _LAP) and np.array_equal(gauss, _GAU)):
        raise NotImplementedError("non-canonical conv kernels")


# canonical LTF: out = [12c + 5a + 3d > 22.5]  over (center, adj, diag),
# i.e. conv3x3(x, [[3,5,3],[5,12,5],[3,5,3]]) > 22.5
PROF_A = (3.0, 5.0, 3.0)       # left/right column vertical profile
PROF_C = (5.0, 12.0, 5.0)      # center column vertical profile
TH = 22                        # bit = [conv > 22.5] = [int conv > 22]


def _band(prof, n=128, block=None):
    m = np.zeros((n, n), np.float32)
    if block is None:
        for om in range(n):
            for dy in range(3):
                p = om + dy - 1
                if 0 <= p < n:
                    m[p, om] = prof[dy]
    else:
        nb = n // block
        for b in range(nb):
            for om in range(block):
                for dy in range(3):
                    p = om + dy - 1
                    if 0 <= p < block:
                        m[b * block + p, b * block + om] = prof[dy]
    return m


PROF_Q0 = tuple(c + a / BASE for c, a in zip(PROF_C, PROF_A))
PROF_Q1 = tuple(a / BASE for a in PROF_A)
PROF_P = tuple(a * BASE for a in PROF_A)


def build_weights():
    """[128, 6*128] fp16: per variant (mid, tail) the three pass bands
    [band(c + a/64) | band(a/64) | band(64a)]."""
    cols = []
    for block in (None, TAIL_S):
        cols += [_band(PROF_Q0, block=block), _band(PROF_Q1, block=block),
                 _band(PROF_P, block=block)]
    return np.concatenate(cols, axis=1).astype(np.float16)


def build_streams(x):
    """x: [B, H, W] binary f32 -> Q [B, H, NP] fp16, P2 [B, H, NP] fp8."""
    import ml_dtypes
    Q = (BASE * x[:, :, 0::2] + x[:, :, 1::2]).astype(np.float16)
    P2 = np.zeros((x.shape[0], x.shape[1], NP), np.float32)
    P2[:, :, 1:] = x[:, :, 1:-2:2]      # x[2c-1] for c >= 1
    P2 += x[:, :, 1::2]                 # + x[2c+1]
    return (np.ascontiguousarray(Q),
            np.ascontiguousarray(P2.astype(ml_dtypes.float8_e4m3)))


def build_nc(lg=8):
    nc = bacc.Bacc()
    q_d = nc.declare_dram_parameter("Q", [B_PER_CORE, H, NP], FP16,
                                    isOutput=False)
    p_d = nc.declare_dram_parameter("P2", [B_PER_CORE, H, NP], FP8,
                                    isOutput=False)
    w_d = nc.declare_dram_parameter("wts", [128, 6 * 128], FP16,
                                    isOutput=False)
    psi_d = nc.declare_dram_parameter("psi", [N_UNITS, 128, NP], U16,
                                      isOutput=True)

    with tile.TileContext(nc) as tc:
        with (
            tc.tile_pool(name="const", bufs=1) as cpool,
            tc.tile_pool(name="xin", bufs=4) as xpool,
            tc.tile_pool(name="oput", bufs=4) as opool,
            tc.tile_pool(name="psum", bufs=8, space="PSUM") as ppool,
        ):
            wsb = cpool.tile([128, 6 * 128], FP16)
            nc.gpsimd.dma_start(wsb[:], w_d[:])

            def conv_unit(qt, pt, variant, out_ap, cvt_engine):
                """3 matmuls -> psi PSUM; convert to uint16 at out_ap."""
                ps = ppool.tile([128, NP], F32, tag="psi")
                w0 = wsb[:, variant * 384:variant * 384 + 128]
                w1 = wsb[:, variant * 384 + 128:variant * 384 + 256]
                wp = wsb[:, variant * 384 + 256:variant * 384 + 384]
                nc.tensor.matmul(ps[:], w0, qt[:, 0:NP], start=True,
                                 stop=False)
                nc.tensor.matmul(ps[:], wp, pt[:, 0:NP], start=False,
                                 stop=False)
                # B-right tap: rider band on Q shifted left by one pair;
                # last pair column has no right neighbour (zero pad).
                nc.tensor.matmul(ps[:, 0:NP - 1], w1, qt[:, 1:NP],
                                 start=False, stop=True)
                if cvt_engine == 0:
                    nc.vector.tensor_scalar(out_ap, ps[:], 0.0, None, Op.add)
                else:
                    nc.scalar.copy(out_ap, ps[:])

            def load_q(b, k0, g):
                qt = xpool.tile([128, g, NP], FP16, tag=f"xq{g}")
                nc.sync.dma_start(
                    qt[:], bass.AP(q_d, (b * H + 126 * k0) * NP,
                                   [[NP, 128], [126 * NP, g], [1, NP]]))
                return qt

            p_ctr = [0]

            def load_p(b, k0, g):
                pt = xpool.tile([128, g, NP], FP8, tag=f"xp{g}")
                # alternate the config cost between the two HWDGE queues
                eng = nc.scalar if p_ctr[0] % 2 == 0 else nc.sync
                p_ctr[0] += 1
                eng.dma_start(
                    pt[:], bass.AP(p_d, (b * H + 126 * k0) * NP,
                                   [[NP, 128], [126 * NP, g], [1, NP]]))
                return pt

            store_ctr = [0]

            def do_units(qt, pt, poff, uidx, g, sg_max=4, alt=False):
                for j0 in range(0, g, sg_max):
                    sg = min(sg_max, g - j0)
                    ut = opool.tile([128, sg, NP], U16, tag="uo")
                    for j in range(sg):
                        conv_unit(qt[:, j0 + j, :], pt[:, poff + j0 + j, :],
                                  0, ut[:, j, :], (uidx + j) % 2)
                    # spread the final store burst over two queues
                    st_eng = nc.sync if (alt and store_ctr[0] % 2)                         else nc.gpsimd
                    store_ctr[0] += 1
                    st_eng.dma_start(
                        bass.AP(psi_d, uidx * 128 * NP,
                                [[NP, 128], [128 * NP, sg], [1, NP]]),
                        ut[:])
                    uidx += sg
                return uidx

            # image 0 in progressively larger groups so the first matmul
            # starts as soon as one unit's data lands
            uidx = 0
            g0 = [(0, 1), (1, 1), (2, 2), (4, 4)]
            tiles0 = [(load_q(0, k0, g), load_p(0, k0, g)) for (k0, g) in g0]

            # tail (rows 1008..1023 of all 8 images): loads issued early,
            # compute hidden inside the pipeline instead of draining at
            # the end
            qtt = xpool.tile([128, NP], FP16, tag="xqt")
            nc.sync.dma_start(qtt[:], q_d[:, H - TAIL_S:H, :])
            ptt = xpool.tile([128, NP], FP8, tag="xpt")
            nc.scalar.dma_start(ptt[:], p_d[:, H - TAIL_S:H, :])

            for (k0, g), (qt, pt) in zip(g0, tiles0):
                uidx = do_units(qt, pt, 0, uidx, g)

            utt = opool.tile([128, NP], U16, tag="uot")
            conv_unit(qtt[:], ptt[:], 1, utt[:], 0)
            nc.gpsimd.dma_start(psi_d[N_UNITS - 1, :, :], utt[:])

            for b in range(1, B_PER_CORE):
                last = b >= B_PER_CORE - 2
                for k0 in range(0, N_MAIN, 4):
                    qt = load_q(b, k0, 4)
                    pt = load_p(b, k0, 4)
                    uidx = do_units(qt, pt, 0, uidx, 4,
                                    2 if last else 4, alt=last)

    return nc


_NC_CACHE = {}


def _get_nc(key=0):
    if key not in _NC_CACHE:
        nc = build_nc()
        nc.finalize()
        _NC_CACHE[key] = nc
    return _NC_CACHE[key]


def decode_psi(psi_all):
    """psi_all: [N_CORES, N_UNITS, 128, NP] uint16 -> [64, H, W] f32."""
    p = psi_all.astype(np.int32)
    ebit = ((p >> 6) > TH)
    obit = ((p & 63) > TH)
    # interleave pairs -> pixel columns
    bits = np.empty(p.shape[:-1] + (W,), np.float32)
    bits[..., 0::2] = ebit
    bits[..., 1::2] = obit
    out = np.empty((N_CORES, B_PER_CORE, H, W), np.float32)
    for b in range(B_PER_CORE):
        for k in range(N_MAIN):
            u = b * N_MAIN + k
            if k == 0:
                out[:, b, 0:127, :] = bits[:, u, 0:127, :]
            else:
                out[:, b, 126 * k + 1:126 * k + 127, :] = bits[:, u, 1:127, :]
        # tail unit: block rows 1..15 -> image rows 1009..1023
        out[:, b, H - TAIL_S + 1:H, :] = \
            bits[:, N_UNITS - 1, b * TAIL_S + 1:(b + 1) * TAIL_S, :]
    return out.reshape(N_CORES * B_PER_CORE, H, W)


def kernel(mask, lap_kernel, gauss_kernel):
    _check_kernels(lap_kernel, gauss_kernel)
    mask = np.asarray(mask, dtype=np.float32)
    bb, cc, h, w = mask.shape
    assert (h, w) == (H, W) and bb * cc == N_CORES * B_PER_CORE
    x_all = np.ascontiguousarray(mask.reshape(N_CORES * B_PER_CORE, h, w))
    wts = build_weights()

    in_maps = []
    for c in range(N_CORES):
        Q, P2 = build_streams(x_all[c * B_PER_CORE:(c + 1) * B_PER_CORE])
        in_maps.append({"Q": Q, "P2": P2, "wts": wts})

    nc = _get_nc()
    res = run_bass_kernel_spmd(nc, in_maps, list(range(N_CORES)))
    psi_all = np.stack([res.results[c]["psi"] for c in range(N_CORES)])
    out = decode_psi(psi_all)
    return out.reshape(bb, cc, h, w).astype(np.float32)


# revision 36
# speedup vs baseline: 1.0654x; 1.0654x over previous
"""Trainium2 Bass kernel for BinaryMaskEdgeSmoothing.

The reference per-pixel computation (two 3x3 convs + sigmoid blend +
threshold) collapses, for binary inputs, to a single linear threshold
function of the 3x3 neighborhood (verified against the f32 reference
over all 512 neighborhoods):

    out = [ conv3x3(x, K) > 22.5 ],  K = [[3,5,3],[5,12,5],[3,5,3]]

Device decomposition (8 NeuronCores, data-parallel over B*C=64 images,
8 images per core, row-tiles of 128 rows with stride 126):

  * Two adjacent pixels are packed into one fp16 element on the host,
    plus a pre-summed odd-pixel fp8 stream (1.5 B/px total input):
      Q[r, c]  = 64*x[r, 2c] + x[r, 2c+1]       (values {0,1,64,65})
      P2[r, c] = x[r, 2c-1] + x[r, 2c+1]        (values {0,1,2}, fp8)
    Three banded-matrix matmuls (TensorEngine; vertical taps in the
    128x128 band, vertical profiles a=[3,5,3], c=[5,12,5]) compute BOTH
    pixels' conv values in one PSUM element:
      psi = band(c + a/64)@Q + band(64a)@P2 + band(a/64)@Q[:, 1:]
          = 64*A + B + eps   (A = conv at even px, B = odd px)
    where eps in [0, 0.35) is a deliberate fractional residue (the a/64
    rider of each Q pass lands on the neighbouring odd pixel) plus
    <~0.1 of fp22 product rounding; both vanish in the uint16 round.
  * psi is converted to uint16 (DVE/ACT alternating) and DMA'd out; the
    host applies both thresholds:
      even bit = (psi >> 6) > 22,  odd bit = (psi & 63) > 22
    (A, B in [0,44], so the bases never interact; arithmetic exact.)
  * Rows 1009..1023 of all 8 images are batched into one final tile
    with block-diagonal band matrices.
"""

import numpy as np

import concourse.bass as bass
import concourse.bacc as bacc
import concourse.mybir as mybir
import concourse.tile as tile
from concourse.bass_utils import run_bass_kernel_spmd

Op = mybir.AluOpType
F32 = mybir.dt.float32
FP16 = mybir.dt.float16
FP8 = mybir.dt.float8e4
U16 = mybir.dt.uint16

N_CORES = 8
B_PER_CORE = 8
H = 1024
W = 1024
NP = W // 2          # pair columns
N_MAIN = 8           # main row-units per image (stride 126, 128 rows)
TAIL_S = 16          # input rows per image in the tail unit
N_UNITS = B_PER_CORE * N_MAIN + 1

BASE = 64.0

# Truth table of the reference computation over neighborhood classes
# (center c, adjacent count a, diagonal count d), derived by running the
# f32 jax reference over all 512 binary 3x3 neighborhoods with the
# canonical lap/gauss kernels. Index = c*25 + a*5 + d.
_TRUTH = np.array([int(ch) for ch in (
    "00000" "00000" "00000" "00011" "01111"   # c=0, a=0..4, d=0..4
    "00001" "00111" "01111" "11111" "11111"   # c=1
)], dtype=np.int64)
_LAP = np.array([[-1., -1., -1.], [-1., 8., -1.], [-1., -1., -1.]],
                np.float32)
_GAU = (np.array([[1., 2., 1.], [2., 4., 2.], [1., 2., 1.]],
                 np.float32) / 16.0)


def _check_kernels(lap, gauss):
    """The LTF below encodes the reference's decisions for the canonical
    lap/gauss kernels (verified per-class through the f32 jax reference,
    including the two classes whose exact value is within 1e-8 of 0.5,
    where f32 rounding decides). Different conv kernels would need a
    re-derived table, so refuse them loudly."""
    lap = np.asarray(lap, np.float32).reshape(3, 3)
    gauss = np.asarray(gauss, np.float32).reshape(3, 3)
    if not (np.array_equal(lap, _LAP) and np.array_equal(gauss, _GAU)):
        raise NotImplementedError("non-canonical conv kernels")


# canonical LTF: out = [12c + 5a + 3d > 22.5]  over (center, adj, diag),
# i.e. conv3x3(x, [[3,5,3],[5,12,5],[3,5,3]]) > 22.5
PROF_A = (3.0, 5.0, 3.0)       # left/right column vertical profile
PROF_C = (5.0, 12.0, 5.0)      # center column vertical profile
TH = 22                        # bit = [conv > 22.5] = [int conv > 22]


def _band(prof, n=128, block=None):
    m = np.zeros((n, n), np.float32)
    if block is None:
        for om in range(n):
            for dy in range(3):
                p = om + dy - 1
                if 0 <= p < n:
                    m[p, om] = prof[dy]
    else:
        nb = n // block
        for b in range(nb):
            for om in range(block):
                for dy in range(3):
                    p = om + dy - 1
                    if 0 <= p < block:
                        m[b * block + p, b * block + om] = prof[dy]
    return m


PROF_Q0 = tuple(c + a / BASE for c, a in zip(PROF_C, PROF_A))
PROF_Q1 = tuple(a / BASE for a in PROF_A)
PROF_P = tuple(a * BASE for a in PROF_A)


def build_weights():
    """[128, 6*128] fp16: per variant (mid, tail) the three pass bands
    [band(c + a/64) | band(a/64) | band(64a)]."""
    cols = []
    for block in (None, TAIL_S):
        cols += [_band(PROF_Q0, block=block), _band(PROF_Q1, block=block),
                 _band(PROF_P, block=block)]
    return np.concatenate(cols, axis=1).astype(np.float16)


def build_streams(x):
    """x: [B, H, W] binary f32 -> Q [B, H, NP] fp16, P2 [B, H, NP] fp8."""
    import ml_dtypes
    Q = (BASE * x[:, :, 0::2] + x[:, :, 1::2]).astype(np.float16)
    P2 = np.zeros((x.shape[0], x.shape[1], NP), np.float32)
    P2[:, :, 1:] = x[:, :, 1:-2:2]      # x[2c-1] for c >= 1
    P2 += x[:, :, 1::2]                 # + x[2c+1]
    return (np.ascontiguousarray(Q),
            np.ascontiguousarray(P2.astype(ml_dtypes.float8_e4m3)))


def build_nc(lg=8):
    nc = bacc.Bacc()
    q_d = nc.declare_dram_parameter("Q", [B_PER_CORE, H, NP], FP16,
                                    isOutput=False)
    p_d = nc.declare_dram_parameter("P2", [B_PER_CORE, H, NP], FP8,
                                    isOutput=False)
    w_d = nc.declare_dram_parameter("wts", [128, 6 * 128], FP16,
                                    isOutput=False)
    psi_d = nc.declare_dram_parameter("psi", [N_UNITS, 128, NP], U16,
                                      isOutput=True)

    with tile.TileContext(nc) as tc:
        with (
            tc.tile_pool(name="const", bufs=1) as cpool,
            tc.tile_pool(name="xin", bufs=4) as xpool,
            tc.tile_pool(name="oput", bufs=4) as opool,
            tc.tile_pool(name="psum", bufs=8, space="PSUM") as ppool,
        ):
            wsb = cpool.tile([128, 6 * 128], FP16)
            nc.gpsimd.dma_start(wsb[:], w_d[:])

            def conv_unit(qt, pt, variant, out_ap, cvt_engine):
                """3 matmuls -> psi PSUM; convert to uint16 at out_ap."""
                ps = ppool.tile([128, NP], F32, tag="psi")
                w0 = wsb[:, variant * 384:variant * 384 + 128]
                w1 = wsb[:, variant * 384 + 128:variant * 384 + 256]
                wp = wsb[:, variant * 384 + 256:variant * 384 + 384]
                nc.tensor.matmul(ps[:], w0, qt[:, 0:NP], start=True,
                                 stop=False)
                nc.tensor.matmul(ps[:], wp, pt[:, 0:NP], start=False,
                                 stop=False)
                # B-right tap: rider band on Q shifted left by one pair;
                # last pair column has no right neighbour (zero pad).
                nc.tensor.matmul(ps[:, 0:NP - 1], w1, qt[:, 1:NP],
                                 start=False, stop=True)
                if cvt_engine == 0:
                    nc.vector.tensor_scalar(out_ap, ps[:], 0.0, None, Op.add)
                else:
                    nc.scalar.copy(out_ap, ps[:])

            def load_q(b, k0, g):
                qt = xpool.tile([128, g, NP], FP16, tag=f"xq{g}")
                nc.sync.dma_start(
                    qt[:], bass.AP(q_d, (b * H + 126 * k0) * NP,
                                   [[NP, 128], [126 * NP, g], [1, NP]]))
                return qt

            def load_p(b, k0, g):
                pt = xpool.tile([128, g, NP], FP8, tag=f"xp{g}")
                nc.scalar.dma_start(
                    pt[:], bass.AP(p_d, (b * H + 126 * k0) * NP,
                                   [[NP, 128], [126 * NP, g], [1, NP]]))
                return pt

            store_ctr = [0]

            def do_units(qt, pt, poff, uidx, g, sg_max=4, alt=False):
                for j0 in range(0, g, sg_max):
                    sg = min(sg_max, g - j0)
                    ut = opool.tile([128, sg, NP], U16, tag="uo")
                    for j in range(sg):
                        conv_unit(qt[:, j0 + j, :], pt[:, poff + j0 + j, :],
                                  0, ut[:, j, :], (uidx + j) % 2)
                    # spread the final store burst over two queues
                    st_eng = nc.sync if (alt and store_ctr[0] % 2)                         else nc.gpsimd
                    store_ctr[0] += 1
                    st_eng.dma_start(
                        bass.AP(psi_d, uidx * 128 * NP,
                                [[NP, 128], [128 * NP, sg], [1, NP]]),
                        ut[:])
                    uidx += sg
                return uidx

            # image 0 in progressively larger groups so the first matmul
            # starts as soon as one unit's data lands
            uidx = 0
            g0 = [(0, 1), (1, 1), (2, 2), (4, 4)]
            # prime the pipeline: spread the first groups' DMA configs
            # across all three queues so unit 0 lands as early as possible
            q_engs = [nc.sync, nc.scalar, nc.sync, nc.sync]
            p_engs = [nc.scalar, nc.gpsimd, nc.scalar, nc.scalar]
            tiles0 = []
            for i, (k0, g) in enumerate(g0):
                qt = xpool.tile([128, g, NP], FP16, tag=f"xq{g}")
                q_engs[i].dma_start(
                    qt[:], bass.AP(q_d, 126 * k0 * NP,
                                   [[NP, 128], [126 * NP, g], [1, NP]]))
                pt = xpool.tile([128, g, NP], FP8, tag=f"xp{g}")
                p_engs[i].dma_start(
                    pt[:], bass.AP(p_d, 126 * k0 * NP,
                                   [[NP, 128], [126 * NP, g], [1, NP]]))
                tiles0.append((qt, pt))

            # tail (rows 1008..1023 of all 8 images): loads issued early,
            # compute hidden inside the pipeline instead of draining at
            # the end
            qtt = xpool.tile([128, NP], FP16, tag="xqt")
            nc.sync.dma_start(qtt[:], q_d[:, H - TAIL_S:H, :])
            ptt = xpool.tile([128, NP], FP8, tag="xpt")
            nc.scalar.dma_start(ptt[:], p_d[:, H - TAIL_S:H, :])

            for (k0, g), (qt, pt) in zip(g0, tiles0):
                uidx = do_units(qt, pt, 0, uidx, g)

            utt = opool.tile([128, NP], U16, tag="uot")
            conv_unit(qtt[:], ptt[:], 1, utt[:], 0)
            nc.gpsimd.dma_start(psi_d[N_UNITS - 1, :, :], utt[:])

            for b in range(1, B_PER_CORE):
                last = b >= B_PER_CORE - 2
                for k0 in range(0, N_MAIN, 4):
                    qt = load_q(b, k0, 4)
                    pt = load_p(b, k0, 4)
                    uidx = do_units(qt, pt, 0, uidx, 4,
                                    2 if last else 4, alt=last)

    return nc


_NC_CACHE = {}


def _get_nc(key=0):
    if key not in _NC_CACHE:
        nc = build_nc()
        nc.finalize()
        _NC_CACHE[key] = nc
    return _NC_CACHE[key]


def decode_psi(psi_all):
    """psi_all: [N_CORES, N_UNITS, 128, NP] uint16 -> [64, H, W] f32."""
    p = psi_all.astype(np.int32)
    ebit = ((p >> 6) > TH)
    obit = ((p & 63) > TH)
    # interleave pairs -> pixel columns
    bits = np.empty(p.shape[:-1] + (W,), np.float32)
    bits[..., 0::2] = ebit
    bits[..., 1::2] = obit
    out = np.empty((N_CORES, B_PER_CORE, H, W), np.float32)
    for b in range(B_PER_CORE):
        for k in range(N_MAIN):
            u = b * N_MAIN + k
            if k == 0:
                out[:, b, 0:127, :] = bits[:, u, 0:127, :]
            else:
                out[:, b, 126 * k + 1:126 * k + 127, :] = bits[:, u, 1:127, :]
        # tail unit: block rows 1..15 -> image rows 1009..1023
        out[:, b, H - TAIL_S + 1:H, :] = \
            bits[:, N_UNITS - 1, b * TAIL_S + 1:(b + 1) * TAIL_S, :]
    return out.reshape(N_CORES * B_PER_CORE, H, W)


def kernel(mask, lap_kernel, gauss_kernel):
    _check_kernels(lap_kernel, gauss_kernel)
    mask = np.asarray(mask, dtype=np.float32)
    bb, cc, h, w = mask.shape
    assert (h, w) == (H, W) and bb * cc == N_CORES * B_PER_CORE
    x_all = np.ascontiguousarray(mask.reshape(N_CORES * B_PER_CORE, h, w))
    wts = build_weights()

    in_maps = []
    for c in range(N_CORES):
        Q, P2 = build_streams(x_all[c * B_PER_CORE:(c + 1) * B_PER_CORE])
        in_maps.append({"Q": Q, "P2": P2, "wts": wts})

    nc = _get_nc()
    res = run_bass_kernel_spmd(nc, in_maps, list(range(N_CORES)))
    psi_all = np.stack([res.results[c]["psi"] for c in range(N_CORES)])
    out = decode_psi(psi_all)
    return out.reshape(bb, cc, h, w).astype(np.float32)
